# revision 32
# baseline (speedup 1.0000x reference)
"""Trainium2 Bass kernel for the AdapterController hard-routing MoE adapter.

Reference computation (per router m in [0,4), batch b in [0,16)):
    e = expert_index[m, b]
    z = x[b] @ down_w[m, e] + down_b[m, e]      # [512, 256]
    z = z * sigmoid(z)                          # swish
    u = z @ up_w[m, e]                          # [512, 1024]
    out[m, b] = u

Strategy: data-parallel over the batch axis (2 batches per core, 8 cores).
The expert gather is part of input sharding: each core receives exactly the
(m, b)-selected weight matrices, packed on the host into the SBUF partition
layout so every DMA is fully contiguous.

On-chip per (m, b) pair:
    zT[d, s] = sum_c Wd[c, d] * xT[c, s]        (16 matmuls N=512, K=128)
    z = silu(zT + bd)                           (ACT engine, PSUM -> SBUF)
    u[s, c] = sum_d zT[d, s].T @ Wu[d, c]       (16 matmuls N=512)

Schedule notes (final): the matmul stream runs at the warm-PE roofline
(~216 ns per N=512 matmul, 54.6 us total), so the schedule optimizes the
edges.  Measured HW model this schedule is built around:
  - each HWDGE engine (sync=SP, scalar=ACT) owns 4 hardware queues; every
    un-blocked dma_start activates a queue ~0.6us after the previous cfg
    on the same sequencer, and all ACTIVE queues share the 16 DMA engines
    (~360 GB/s) round-robin.  Transfers on one engine do NOT serialize, so
    arrival order is set by cfg-stagger + fair sharing; un-paced transfers
    dilute the stream-critical head bytes.
  - the framework hoists the first act-table load (~1.3us) to the front of
    the scalar stream, so scalar's first cfg fires ~1.3us after sync's.
    All stream-critical halves (x00/wd0/x01) therefore ride sync in exact
    deadline order; scalar carries bias/wu0 (deadlines T0+2..4us), then a
    dummy silu pins the second table load before wd1/wu1/wu2's cfgs.
  - steady state: wd(q) cfg paced behind silu-j0 on the scalar sequencer,
    wu(q) paced behind the a2-out cfg on the sync sequencer, x(b1) behind
    pair 0's first out cfg.  Outs: a0/a2 via sync, a1/a3 via scalar.  No
    pool/SWDGE DMAs (their queue teardown lengthens the epilogue).
  - the PE HAM clock gate needs ~3.4us of matmul activity to lift the
    1.2GHz cold throttle and an idle gap can restart the ramp: the warm-up
    burst (N=128 matmuls) is sized to bridge until first data (~13us).
  - the bias transfer is padded to 576B rows (rows below the 512B SDMA
    line-rate minimum degrade into read-modify-write descriptors).
  - the first pair runs its down-projection k-outer so both PSUM groups
    chase each arriving x chunk.
  - the last pair reorders its up-projection so a2 completes first; the
    final a3 block is h-major with its PSUM taken from the (by then idle)
    psz pool so its matmuls never wait on an output copy; final copies
    split vector/scalar and the outs drain on both HWDGE rings.
"""

import numpy as np

M, B, S, C, D = 4, 16, 512, 1024, 256
N_CORES = 8
B_LOC = B // N_CORES  # batches per core
KC = C // 128         # 8 c-chunks
KD = D // 128         # 2 d-chunks
NPAIR = M * B_LOC     # 8 (m, b) pairs per core
WARM = 52             # warm-up matmuls (N=128) bridging until first data

_cache = {}
last_results = None  # BassKernelResults of the most recent run (for test.py)


def _build():
    from contextlib import ExitStack

    import concourse.mybir as mybir
    import concourse.tile as tile
    from concourse import bacc
    f32 = mybir.dt.float32
    bf16 = mybir.dt.bfloat16
    mm_dt = bf16
    out_dt = bf16

    nc = bacc.Bacc("TRN2", target_bir_lowering=False, debug=False,
                   num_devices=N_CORES)
    # xtp[b, half][p, k*512 + s] = x[b, s, 128*(4*half + k) + p]
    xtp = nc.dram_tensor("xtp", [B_LOC, 2, 128, KC * S // 2], bf16,
                         kind="ExternalInput").ap()
    # wdp[m, b][p, k*256 + d] = down_w_gathered[m, b, 128k + p, d]
    wdp = nc.dram_tensor("wdp", [M, B_LOC, 128, KC * D], bf16,
                         kind="ExternalInput").ap()
    # bdp[p, (m*B_LOC+b)*2 + j] = down_b_gathered[m, b, 128j + p]; padded to
    # 144 f32 columns so each DMA row is 576B (>= the 512B SDMA line-rate
    # minimum)
    bdp = nc.dram_tensor("bdp", [128, 144], f32, kind="ExternalInput").ap()
    # wup[m, b][p, j*1024 + c] = up_w_gathered[m, b, 128j + p, c]
    wup = nc.dram_tensor("wup", [M, B_LOC, 128, KD * C], bf16,
                         kind="ExternalInput").ap()
    out = nc.dram_tensor("out", [M, B_LOC, S, C], out_dt,
                         kind="ExternalOutput").ap()

    silu = mybir.ActivationFunctionType.Silu
    copy_fn = mybir.ActivationFunctionType.Copy

    with tile.TileContext(nc) as tc, ExitStack() as ctx:
        const = ctx.enter_context(tc.tile_pool(name="const", bufs=1))
        xpool = ctx.enter_context(tc.tile_pool(name="xpool", bufs=4))
        wdpool = ctx.enter_context(tc.tile_pool(name="wdpool", bufs=4))
        wupool = ctx.enter_context(tc.tile_pool(name="wupool", bufs=4))
        zpool = ctx.enter_context(tc.tile_pool(name="zpool", bufs=2))
        upool = ctx.enter_context(tc.tile_pool(name="upool", bufs=12))
        pszp = ctx.enter_context(tc.tile_pool(name="pszp", bufs=2, space="PSUM"))
        psup = ctx.enter_context(tc.tile_pool(name="psup", bufs=3, space="PSUM"))

        bd_sb = const.tile([128, 144], f32)

        # PE warm-up: the HAM clock gate needs ~3.4us of uninterrupted PE
        # activity to lift the 1.2GHz cold throttle, and an idle gap restarts
        # the ramp; the burst is sized to bridge from program start until the
        # first weights/x land (~12us)
        warm_src = const.tile([128, 128], mm_dt)
        nc.gpsimd.memset(warm_src[:], 0)
        warm_ps = pszp.tile([128, 128], f32, tag="psz", name="warm_ps")
        for _ in range(WARM):
            nc.tensor.matmul(warm_ps[:], warm_src[:], warm_src[:],
                             start=True, stop=True)

        # Each DMA ring drains FIFO in trigger order, so per-ring emission
        # order pins byte-arrival order. (No dep-chaining between DMAs: a dep
        # on a DMA instruction waits for its data semaphore, which would
        # serialize transfer-after-completion.)
        xh = {b: [xpool.tile([128, KC * S // 2], mm_dt, tag="xt",
                             name=f"xt_{b}_{h}") for h in range(2)]
              for b in range(B_LOC)}
        wd_t = {p: wdpool.tile([128, KC * D], mm_dt, tag="wd",
                               name=f"wd{p}")
                for p in range(2)}
        wu_t = {p: wupool.tile([128, KD * C], mm_dt, tag="wu", name=f"wu{p}")
                for p in range(2)}

        # Head fill.  HW model: each HWDGE engine (sync=SP, scalar=ACT) owns
        # 4 hardware queues; every un-blocked dma_start costs ~0.6us on its
        # sequencer AND ~0.63us on the single shared HWDGE descriptor
        # generator, then all ACTIVE queues share the 16 DMA engines
        # (~360 GB/s) round-robin.  Transfers on one engine do NOT
        # serialize; arrival order is set by cfg-stagger + fair sharing.
        # The stream-critical bytes (wd0 + x00, then x01) are split across
        # BOTH engines' first queues; the later scalar cfgs are staggered
        # behind them (and behind the act-table loads) so they cannot steal
        # much head bandwidth.
        # All stream-critical halves ride SYNC in exact deadline order (its
        # first cfg fires ~1.3us earlier than scalar's, which sits behind
        # the hoisted act-table load); scalar only carries transfers whose
        # deadlines are ~T0+2..4us.
        nc.sync.dma_start(xh[0][0][:, :1024], xtp[0, 0][:, :1024])    # x00a
        nc.sync.dma_start(wd_t[0][:, :1024], wdp[0, 0][:, :1024])     # wd0a
        nc.sync.dma_start(xh[0][0][:, 1024:], xtp[0, 0][:, 1024:])    # x00b
        nc.sync.dma_start(wd_t[0][:, 1024:], wdp[0, 0][:, 1024:])     # wd0b
        nc.sync.dma_start(xh[0][1][:, :1024], xtp[0, 1][:, :1024])    # x01a
        nc.sync.dma_start(xh[0][1][:, 1024:], xtp[0, 1][:, 1024:])    # x01b
        nc.scalar.dma_start(bd_sb[:], bdp[:])        # bias (576B rows)
        nc.scalar.dma_start(wu_t[0][:, :1024], wup[0, 0][:, :1024])  # wu0-j0
        nc.scalar.dma_start(wu_t[0][:, 1024:], wup[0, 0][:, 1024:])  # wu0-j1
        # dummy silu: forces the ~1.3us act-table load HERE (scalar seq),
        # which also delays the wd1/wu1/wu2 cfgs below past the head burst
        dummy_sb = const.tile([128, 8], mm_dt)
        nc.scalar.activation(dummy_sb[:], warm_src[:, 0:8], silu)
        nc.scalar.dma_start(wd_t[1][:], wdp[1, 0])   # wd1
        nc.scalar.dma_start(wu_t[1][:], wup[1, 0])   # wu1
        wu_t[2] = wupool.tile([128, KD * C], mm_dt, tag="wu", name="wu2")
        nc.scalar.dma_start(wu_t[2][:], wup[2 % M, 2 // M])   # wu2
        # x(b1) rides the pool ring but is emitted after pair 0's first out
        # trigger (see the pair loop), so the pool FIFO holds it back until
        # ~T0+4us -- long before x(b1) is consumed (pair 4) and without
        # stealing head bandwidth from wd0/x(b0).

        for p in range(NPAIR):
            m, b = p % M, p // M
            q = p + 2
            if q < NPAIR:
                wd_t[q] = wdpool.tile([128, KC * D], mm_dt, tag="wd",
                                      name=f"wd{q}")
            q3 = p + 3
            if 3 <= q3 < NPAIR:
                wu_t[q3] = wupool.tile([128, KD * C], mm_dt, tag="wu",
                                       name=f"wu{q3}")

            wd_sb, wu_sb, xb = wd_t[p], wu_t[p], xh[b]
            z_sb = zpool.tile([128, KD, S], mm_dt)
            if p == 0:
                # Hybrid order for the first pair: k-outer over k0-3 so both
                # PSUM groups chase the arriving x00 chunk (halving the early
                # x-consumption rate), then j-outer over k4-7 so psz-j0 stops
                # 4 matmuls before the down phase ends and silu j0 (+ its
                # ~0.7us latency) finishes right as the up phase needs z-j0.
                psz_j = [pszp.tile([128, S], f32, tag="psz", name=f"psz{j}")
                         for j in range(KD)]
                for k in range(KC // 2):
                    for j in range(KD):
                        nc.tensor.matmul(
                            psz_j[j][:],
                            wd_sb[:, k * 256 + j * 128:
                                  k * 256 + j * 128 + 128],
                            xb[k // 4][:, (k % 4) * S: (k % 4 + 1) * S],
                            start=(k == 0), stop=False,
                        )
                for j in range(KD):
                    for k in range(KC // 2, KC):
                        nc.tensor.matmul(
                            psz_j[j][:],
                            wd_sb[:, k * 256 + j * 128:
                                  k * 256 + j * 128 + 128],
                            xb[k // 4][:, (k % 4) * S: (k % 4 + 1) * S],
                            start=False, stop=(k == KC - 1),
                        )
                    col = (m * B_LOC + b) * KD + j
                    nc.scalar.activation(z_sb[:, j, :], psz_j[j][:], silu,
                                         bias=bd_sb[:, col: col + 1])
                    if j == 0 and q < NPAIR:
                        # wd(q) cfg is paced behind silu j0 on the scalar
                        # sequencer: it can't fire before ~T0+2us, keeping
                        # the head queues clear for the critical bytes
                        nc.scalar.dma_start(wd_t[q][:], wdp[q % M, q // M])
            else:
                for j in range(KD):
                    psz = pszp.tile([128, S], f32)
                    for k in range(KC):
                        nc.tensor.matmul(
                            psz[:],
                            wd_sb[:, k * 256 + j * 128:
                                  k * 256 + j * 128 + 128],
                            xb[k // 4][:, (k % 4) * S: (k % 4 + 1) * S],
                            start=(k == 0), stop=(k == KC - 1),
                        )
                    col = (m * B_LOC + b) * KD + j
                    nc.scalar.activation(z_sb[:, j, :], psz[:], silu,
                                         bias=bd_sb[:, col: col + 1])
                    if j == 0 and q < NPAIR:
                        nc.scalar.dma_start(wd_t[q][:], wdp[q % M, q // M])

            # one 2-bank PSUM tile per a holds a full [128, 1024] u row;
            # groups are interleaved so j=1 matmuls trail the j=1 silu by a
            # couple of matmul slots (no PE stall).
            last = p == NPAIR - 1
            if not last:
                order = ((0, 0), (0, 1), (1, 0), (1, 1),
                         (0, 2), (0, 3), (1, 2), (1, 3))
                psu_by_a = {}
                for j, a in order:
                    if j == 0:
                        psu_by_a[a] = psup.tile([128, C], f32, tag="psu",
                                                name=f"psu_{p}_{a}")
                    psu = psu_by_a[a]
                    for h in range(2):
                        nc.tensor.matmul(
                            psu[:, h * 512: (h + 1) * 512],
                            z_sb[:, j, a * 128: (a + 1) * 128],
                            wu_sb[:, j * 1024 + h * 512:
                                  j * 1024 + h * 512 + 512],
                            start=(j == 0), stop=(j == KD - 1),
                            skip_group_check=True,
                        )
                    if j == KD - 1:
                        u_sb = upool.tile([128, C], out_dt, tag="u")
                        orow = out[m, b, a * 128:(a + 1) * 128, :]
                        if a % 2 == 0:
                            nc.vector.tensor_copy(u_sb[:], psu[:])
                        else:
                            nc.scalar.activation(u_sb[:], psu[:], copy_fn)
                        # outs: a0/a2 on sync, a1/a3 on scalar -- no pool
                        # DMAs anywhere (SWDGE use costs ~1.5us of epilogue
                        # queue teardown)
                        if a in (0, 2):
                            nc.sync.dma_start(orow, u_sb[:])
                            if p == 0 and a == 0:
                                # x(b1): behind the (blocking) first out cfg
                                # on the sync sequencer, so it can't steal
                                # head bandwidth
                                nc.sync.dma_start(xh[1][0][:], xtp[1, 0])
                                nc.sync.dma_start(xh[1][1][:], xtp[1, 1])
                            if a == 2 and 3 <= p + 3 < NPAIR:
                                # wu(p+3) cfg paced behind the a2 out
                                # trigger on the sync sequencer
                                nc.sync.dma_start(wu_t[p + 3][:],
                                                 wup[(p + 3) % M,
                                                     (p + 3) // M])
                        else:
                            nc.scalar.dma_start(orow, u_sb[:])

            else:
                # Final pair: a2 completes first, then a0, a1; a3 runs
                # h-major so its first half can be copied + DMA'd while the
                # second half's matmuls run.  Copies split vector/scalar so
                # neither engine's chain extends past the last matmul by
                # more than one half-copy; out cfgs spread over sync (3) and
                # pool (3) so no sequencer serializes more than ~3 cfgs.
                order = ((0, 2), (0, 0), (1, 2), (1, 0), (0, 1), (1, 1))
                psu_by_a = {}
                u_by_a = {}
                for j, a in order:
                    if j == 0:
                        psu_by_a[a] = psup.tile([128, C], f32, tag="psu",
                                                name=f"psu_{p}_{a}")
                    psu = psu_by_a[a]
                    for h in range(2):
                        nc.tensor.matmul(
                            psu[:, h * 512: (h + 1) * 512],
                            z_sb[:, j, a * 128: (a + 1) * 128],
                            wu_sb[:, j * 1024 + h * 512:
                                  j * 1024 + h * 512 + 512],
                            start=(j == 0), stop=(j == KD - 1),
                            skip_group_check=True,
                        )
                    if j == KD - 1:
                        u_sb = upool.tile([128, C], out_dt, tag="u",
                                          name=f"u_last_{a}")
                        u_by_a[a] = u_sb
                        orow = out[m, b, a * 128:(a + 1) * 128, :]
                        if a == 2:
                            nc.vector.tensor_copy(u_sb[:], psu[:])
                            nc.sync.dma_start(orow, u_sb[:])
                        elif a == 0:
                            nc.scalar.activation(u_sb[:], psu[:], copy_fn)
                            nc.scalar.dma_start(orow, u_sb[:])
                        else:  # a == 1: halves on both engines
                            nc.vector.tensor_copy(u_sb[:, :512], psu[:, :512])
                            nc.sync.dma_start(orow[:, :512], u_sb[:, :512])
                            nc.scalar.activation(u_sb[:, 512:], psu[:, 512:],
                                                 copy_fn)
                            nc.scalar.dma_start(orow[:, 512:], u_sb[:, 512:])
                # a3: h-major accumulation so h0 completes 2 matmuls early.
                # Its PSUM comes from the pszp pool (free once the silus are
                # done) so the a3 matmuls never wait on an output copy.
                for h in range(2):
                    psu3 = pszp.tile([128, 512], f32, tag="psz",
                                     name=f"psu3h{h}")
                    for j in range(KD):
                        nc.tensor.matmul(
                            psu3[:],
                            z_sb[:, j, 3 * 128: 4 * 128],
                            wu_sb[:, j * 1024 + h * 512:
                                  j * 1024 + h * 512 + 512],
                            start=(j == 0), stop=(j == KD - 1),
                        )
                    u3 = upool.tile([128, C // 2], out_dt, tag="u",
                                    name=f"u_last_3{h}")
                    orow3 = out[m, b, 3 * 128: 4 * 128, :]
                    if h == 0:
                        nc.vector.tensor_copy(u3[:], psu3[:])
                        nc.sync.dma_start(orow3[:, :512], u3[:])
                    else:
                        nc.scalar.activation(u3[:], psu3[:], copy_fn)
                        nc.scalar.dma_start(orow3[:, 512:], u3[:])

    nc.compile()
    return nc


def _get_nc():
    if "nc" not in _cache:
        _cache["nc"] = _build()
    return _cache["nc"]


def kernel(x, expert_index, down_w, down_b, up_w):
    global last_results
    import ml_dtypes
    from concourse import bass_utils

    x = np.asarray(x, dtype=np.float32)
    idx = np.asarray(expert_index)
    r = np.arange(M)[:, None]
    wd = np.asarray(down_w, dtype=np.float32)[r, idx]   # [M, B, C, D]
    bd = np.asarray(down_b, dtype=np.float32)[r, idx]   # [M, B, D]
    wu = np.asarray(up_w, dtype=np.float32)[r, idx]     # [M, B, D, C]

    # Pack into SBUF partition-major layouts (see _build comments).
    xt = x.transpose(0, 2, 1).reshape(B, 2, KC // 2, 128, S)
    xt = xt.transpose(0, 1, 3, 2, 4).reshape(B, 2, 128, KC * S // 2)
    wdp = wd.reshape(M, B, KC, 128, D).transpose(0, 1, 3, 2, 4)
    wdp = wdp.reshape(M, B, 128, KC * D)
    wup = wu.reshape(M, B, KD, 128, C).transpose(0, 1, 3, 2, 4)
    wup = wup.reshape(M, B, 128, KD * C)
    bdp = bd.reshape(M, B, KD, 128).transpose(3, 0, 1, 2)  # [128, M, B, KD]

    in_dt = ml_dtypes.bfloat16

    in_maps = []
    for i in range(N_CORES):
        bs = slice(i * B_LOC, (i + 1) * B_LOC)
        # bias rows padded to 576B (see _build): cols 0:16 real, rest zero
        bias_pad = np.zeros((128, 144), dtype=np.float32)
        bias_pad[:, :M * B_LOC * KD] = \
            bdp[:, :, bs, :].reshape(128, M * B_LOC * KD)
        in_maps.append({
            "xtp": np.ascontiguousarray(xt[bs].astype(in_dt)),
            "wdp": np.ascontiguousarray(wdp[:, bs].astype(in_dt)),
            "wup": np.ascontiguousarray(wup[:, bs].astype(in_dt)),
            "bdp": bias_pad,
        })

    nc = _get_nc()
    res = None
    for attempt in range(3):
        try:
            res = bass_utils.run_bass_kernel_spmd(nc, in_maps,
                                                  core_ids=list(range(N_CORES)))
            break
        except Exception:
            # transient NRT_EXEC_UNIT_UNRECOVERABLE device hiccups recover
            # after a short wait; re-raise if persistent
            if attempt == 2:
                raise
            import time
            time.sleep(15)
    last_results = res

    full = np.empty((M, B, S, C), dtype=np.float32)
    for i in range(N_CORES):
        full[:, i * B_LOC:(i + 1) * B_LOC] = np.asarray(
            res.results[i]["out"]).astype(np.float32)
    return full


# revision 33
# speedup vs baseline: 1.0122x; 1.0122x over previous
"""Trainium2 Bass kernel for the AdapterController hard-routing MoE adapter.

Reference computation (per router m in [0,4), batch b in [0,16)):
    e = expert_index[m, b]
    z = x[b] @ down_w[m, e] + down_b[m, e]      # [512, 256]
    z = z * sigmoid(z)                          # swish
    u = z @ up_w[m, e]                          # [512, 1024]
    out[m, b] = u

Strategy: data-parallel over the batch axis (2 batches per core, 8 cores).
The expert gather is part of input sharding: each core receives exactly the
(m, b)-selected weight matrices, packed on the host into the SBUF partition
layout so every DMA is fully contiguous.

On-chip per (m, b) pair:
    zT[d, s] = sum_c Wd[c, d] * xT[c, s]        (16 matmuls N=512, K=128)
    z = silu(zT + bd)                           (ACT engine, PSUM -> SBUF)
    u[s, c] = sum_d zT[d, s].T @ Wu[d, c]       (16 matmuls N=512)

Schedule notes (final): the matmul stream runs at the warm-PE roofline
(~216 ns per N=512 matmul, 54.6 us total), so the schedule optimizes the
edges.  Measured HW model this schedule is built around:
  - each HWDGE engine (sync=SP, scalar=ACT) owns 4 hardware queues; every
    un-blocked dma_start activates a queue ~0.6us after the previous cfg
    on the same sequencer, and all ACTIVE queues share the 16 DMA engines
    (~360 GB/s) round-robin.  Transfers on one engine do NOT serialize, so
    arrival order is set by cfg-stagger + fair sharing; un-paced transfers
    dilute the stream-critical head bytes.
  - the framework hoists the first act-table load (~1.3us) to the front of
    the scalar stream, so scalar's first cfg fires ~1.3us after sync's.
    All stream-critical halves (x00/wd0/x01) therefore ride sync in exact
    deadline order; scalar carries bias/wu0 (deadlines T0+2..4us), then a
    dummy silu pins the second table load before wd1/wu1/wu2's cfgs.
  - steady state: wd(q) cfg paced behind silu-j0 on the scalar sequencer,
    wu(q) paced behind the a2-out cfg on the sync sequencer, x(b1) behind
    pair 0's first out cfg.  Outs: a0/a2 via sync, a1/a3 via scalar.  No
    pool/SWDGE DMAs (their queue teardown lengthens the epilogue).
  - the PE HAM clock gate needs ~3.4us of matmul activity to lift the
    1.2GHz cold throttle and an idle gap can restart the ramp: the warm-up
    burst (N=128 matmuls) is sized to bridge until first data (~13us).
  - the bias transfer is padded to 576B rows (rows below the 512B SDMA
    line-rate minimum degrade into read-modify-write descriptors).
  - the first pair runs its down-projection k-outer so both PSUM groups
    chase each arriving x chunk.
  - the last pair reorders its up-projection so a2 completes first; the
    final a3 block is h-major with its PSUM taken from the (by then idle)
    psz pool so its matmuls never wait on an output copy; final copies
    split vector/scalar and the outs drain on both HWDGE rings.
"""

import numpy as np

M, B, S, C, D = 4, 16, 512, 1024, 256
N_CORES = 8
B_LOC = B // N_CORES  # batches per core
KC = C // 128         # 8 c-chunks
KD = D // 128         # 2 d-chunks
NPAIR = M * B_LOC     # 8 (m, b) pairs per core
WARM = 64             # warm-up matmuls (N=128) bridging until first data

_cache = {}
last_results = None  # BassKernelResults of the most recent run (for test.py)


def _build():
    from contextlib import ExitStack

    import concourse.mybir as mybir
    import concourse.tile as tile
    from concourse import bacc
    f32 = mybir.dt.float32
    bf16 = mybir.dt.bfloat16
    mm_dt = bf16
    out_dt = bf16

    nc = bacc.Bacc("TRN2", target_bir_lowering=False, debug=False,
                   num_devices=N_CORES)
    # xtp[b, half][p, k*512 + s] = x[b, s, 128*(4*half + k) + p]
    xtp = nc.dram_tensor("xtp", [B_LOC, 2, 128, KC * S // 2], bf16,
                         kind="ExternalInput").ap()
    # wdp[m, b][p, k*256 + d] = down_w_gathered[m, b, 128k + p, d]
    wdp = nc.dram_tensor("wdp", [M, B_LOC, 128, KC * D], bf16,
                         kind="ExternalInput").ap()
    # bdp[p, (m*B_LOC+b)*2 + j] = down_b_gathered[m, b, 128j + p]; padded to
    # 144 f32 columns so each DMA row is 576B (>= the 512B SDMA line-rate
    # minimum)
    bdp = nc.dram_tensor("bdp", [128, 144], f32, kind="ExternalInput").ap()
    # wup[m, b][p, j*1024 + c] = up_w_gathered[m, b, 128j + p, c]
    wup = nc.dram_tensor("wup", [M, B_LOC, 128, KD * C], bf16,
                         kind="ExternalInput").ap()
    out = nc.dram_tensor("out", [M, B_LOC, S, C], out_dt,
                         kind="ExternalOutput").ap()

    silu = mybir.ActivationFunctionType.Silu
    copy_fn = mybir.ActivationFunctionType.Copy

    with tile.TileContext(nc) as tc, ExitStack() as ctx:
        const = ctx.enter_context(tc.tile_pool(name="const", bufs=1))
        xpool = ctx.enter_context(tc.tile_pool(name="xpool", bufs=4))
        wdpool = ctx.enter_context(tc.tile_pool(name="wdpool", bufs=4))
        wupool = ctx.enter_context(tc.tile_pool(name="wupool", bufs=4))
        zpool = ctx.enter_context(tc.tile_pool(name="zpool", bufs=2))
        upool = ctx.enter_context(tc.tile_pool(name="upool", bufs=12))
        pszp = ctx.enter_context(tc.tile_pool(name="pszp", bufs=2, space="PSUM"))
        psup = ctx.enter_context(tc.tile_pool(name="psup", bufs=3, space="PSUM"))

        bd_sb = const.tile([128, 144], f32)

        # PE warm-up: the HAM clock gate needs ~3.4us of uninterrupted PE
        # activity to lift the 1.2GHz cold throttle, and an idle gap restarts
        # the ramp; the burst is sized to bridge from program start until the
        # first weights/x land (~12us)
        warm_src = const.tile([128, 128], mm_dt)
        nc.gpsimd.memset(warm_src[:], 0)
        warm_ps = pszp.tile([128, 128], f32, tag="psz", name="warm_ps")
        for _ in range(WARM):
            nc.tensor.matmul(warm_ps[:], warm_src[:], warm_src[:],
                             start=True, stop=True)

        # Each DMA ring drains FIFO in trigger order, so per-ring emission
        # order pins byte-arrival order. (No dep-chaining between DMAs: a dep
        # on a DMA instruction waits for its data semaphore, which would
        # serialize transfer-after-completion.)
        xh = {b: [xpool.tile([128, KC * S // 2], mm_dt, tag="xt",
                             name=f"xt_{b}_{h}") for h in range(2)]
              for b in range(B_LOC)}
        wd_t = {p: wdpool.tile([128, KC * D], mm_dt, tag="wd",
                               name=f"wd{p}")
                for p in range(2)}
        wu_t = {p: wupool.tile([128, KD * C], mm_dt, tag="wu", name=f"wu{p}")
                for p in range(2)}

        # Head fill.  HW model: each HWDGE engine (sync=SP, scalar=ACT) owns
        # 4 hardware queues; every un-blocked dma_start costs ~0.6us on its
        # sequencer AND ~0.63us on the single shared HWDGE descriptor
        # generator, then all ACTIVE queues share the 16 DMA engines
        # (~360 GB/s) round-robin.  Transfers on one engine do NOT
        # serialize; arrival order is set by cfg-stagger + fair sharing.
        # The stream-critical bytes (wd0 + x00, then x01) are split across
        # BOTH engines' first queues; the later scalar cfgs are staggered
        # behind them (and behind the act-table loads) so they cannot steal
        # much head bandwidth.
        # All stream-critical halves ride SYNC in exact deadline order (its
        # first cfg fires ~1.3us earlier than scalar's, which sits behind
        # the hoisted act-table load); scalar only carries transfers whose
        # deadlines are ~T0+2..4us.
        nc.sync.dma_start(xh[0][0][:, :1024], xtp[0, 0][:, :1024])    # x00a
        nc.sync.dma_start(wd_t[0][:, :1024], wdp[0, 0][:, :1024])     # wd0a
        nc.sync.dma_start(xh[0][0][:, 1024:], xtp[0, 0][:, 1024:])    # x00b
        nc.sync.dma_start(wd_t[0][:, 1024:], wdp[0, 0][:, 1024:])     # wd0b
        nc.sync.dma_start(xh[0][1][:, :1024], xtp[0, 1][:, :1024])    # x01a
        nc.sync.dma_start(xh[0][1][:, 1024:], xtp[0, 1][:, 1024:])    # x01b
        nc.scalar.dma_start(bd_sb[:], bdp[:])        # bias (576B rows)
        nc.scalar.dma_start(wu_t[0][:, :1024], wup[0, 0][:, :1024])  # wu0-j0
        nc.scalar.dma_start(wu_t[0][:, 1024:], wup[0, 0][:, 1024:])  # wu0-j1
        # dummy silu: forces the ~1.3us act-table load HERE (scalar seq),
        # which also delays the wd1/wu1/wu2 cfgs below past the head burst
        dummy_sb = const.tile([128, 8], mm_dt)
        nc.scalar.activation(dummy_sb[:], warm_src[:, 0:8], silu)
        nc.scalar.dma_start(wd_t[1][:], wdp[1, 0])   # wd1
        nc.scalar.dma_start(wu_t[1][:], wup[1, 0])   # wu1
        wu_t[2] = wupool.tile([128, KD * C], mm_dt, tag="wu", name="wu2")
        nc.scalar.dma_start(wu_t[2][:], wup[2 % M, 2 // M])   # wu2
        # x(b1) rides the pool ring but is emitted after pair 0's first out
        # trigger (see the pair loop), so the pool FIFO holds it back until
        # ~T0+4us -- long before x(b1) is consumed (pair 4) and without
        # stealing head bandwidth from wd0/x(b0).

        for p in range(NPAIR):
            m, b = p % M, p // M
            q = p + 2
            if q < NPAIR:
                wd_t[q] = wdpool.tile([128, KC * D], mm_dt, tag="wd",
                                      name=f"wd{q}")
            q3 = p + 3
            if 3 <= q3 < NPAIR:
                wu_t[q3] = wupool.tile([128, KD * C], mm_dt, tag="wu",
                                       name=f"wu{q3}")

            wd_sb, wu_sb, xb = wd_t[p], wu_t[p], xh[b]
            z_sb = zpool.tile([128, KD, S], mm_dt)
            if p == 0:
                # Hybrid order for the first pair: k-outer over k0-3 so both
                # PSUM groups chase the arriving x00 chunk (halving the early
                # x-consumption rate), then j-outer over k4-7 so psz-j0 stops
                # 4 matmuls before the down phase ends and silu j0 (+ its
                # ~0.7us latency) finishes right as the up phase needs z-j0.
                psz_j = [pszp.tile([128, S], f32, tag="psz", name=f"psz{j}")
                         for j in range(KD)]
                for k in range(KC // 2):
                    for j in range(KD):
                        nc.tensor.matmul(
                            psz_j[j][:],
                            wd_sb[:, k * 256 + j * 128:
                                  k * 256 + j * 128 + 128],
                            xb[k // 4][:, (k % 4) * S: (k % 4 + 1) * S],
                            start=(k == 0), stop=False,
                        )
                for j in range(KD):
                    for k in range(KC // 2, KC):
                        nc.tensor.matmul(
                            psz_j[j][:],
                            wd_sb[:, k * 256 + j * 128:
                                  k * 256 + j * 128 + 128],
                            xb[k // 4][:, (k % 4) * S: (k % 4 + 1) * S],
                            start=False, stop=(k == KC - 1),
                        )
                    col = (m * B_LOC + b) * KD + j
                    nc.scalar.activation(z_sb[:, j, :], psz_j[j][:], silu,
                                         bias=bd_sb[:, col: col + 1])
                    if j == 0 and q < NPAIR:
                        # wd(q) cfg is paced behind silu j0 on the scalar
                        # sequencer: it can't fire before ~T0+2us, keeping
                        # the head queues clear for the critical bytes
                        nc.scalar.dma_start(wd_t[q][:], wdp[q % M, q // M])
            else:
                for j in range(KD):
                    psz = pszp.tile([128, S], f32)
                    for k in range(KC):
                        nc.tensor.matmul(
                            psz[:],
                            wd_sb[:, k * 256 + j * 128:
                                  k * 256 + j * 128 + 128],
                            xb[k // 4][:, (k % 4) * S: (k % 4 + 1) * S],
                            start=(k == 0), stop=(k == KC - 1),
                        )
                    col = (m * B_LOC + b) * KD + j
                    nc.scalar.activation(z_sb[:, j, :], psz[:], silu,
                                         bias=bd_sb[:, col: col + 1])
                    if j == 0 and q < NPAIR:
                        nc.scalar.dma_start(wd_t[q][:], wdp[q % M, q // M])

            # one 2-bank PSUM tile per a holds a full [128, 1024] u row;
            # groups are interleaved so j=1 matmuls trail the j=1 silu by a
            # couple of matmul slots (no PE stall).
            last = p == NPAIR - 1
            if not last:
                order = ((0, 0), (0, 1), (1, 0), (1, 1),
                         (0, 2), (0, 3), (1, 2), (1, 3))
                psu_by_a = {}
                for j, a in order:
                    if j == 0:
                        psu_by_a[a] = psup.tile([128, C], f32, tag="psu",
                                                name=f"psu_{p}_{a}")
                    psu = psu_by_a[a]
                    for h in range(2):
                        nc.tensor.matmul(
                            psu[:, h * 512: (h + 1) * 512],
                            z_sb[:, j, a * 128: (a + 1) * 128],
                            wu_sb[:, j * 1024 + h * 512:
                                  j * 1024 + h * 512 + 512],
                            start=(j == 0), stop=(j == KD - 1),
                            skip_group_check=True,
                        )
                    if j == KD - 1:
                        u_sb = upool.tile([128, C], out_dt, tag="u")
                        orow = out[m, b, a * 128:(a + 1) * 128, :]
                        if a % 2 == 0:
                            nc.vector.tensor_copy(u_sb[:], psu[:])
                        else:
                            nc.scalar.activation(u_sb[:], psu[:], copy_fn)
                        # outs: a0/a2 on sync, a1/a3 on scalar -- no pool
                        # DMAs anywhere (SWDGE use costs ~1.5us of epilogue
                        # queue teardown)
                        if a in (0, 2):
                            nc.sync.dma_start(orow, u_sb[:])
                            if p == 0 and a == 0:
                                # x(b1): behind the (blocking) first out cfg
                                # on the sync sequencer, so it can't steal
                                # head bandwidth
                                nc.sync.dma_start(xh[1][0][:], xtp[1, 0])
                                nc.sync.dma_start(xh[1][1][:], xtp[1, 1])
                            if a == 2 and 3 <= p + 3 < NPAIR:
                                # wu(p+3) cfg paced behind the a2 out
                                # trigger on the sync sequencer
                                nc.sync.dma_start(wu_t[p + 3][:],
                                                 wup[(p + 3) % M,
                                                     (p + 3) // M])
                        else:
                            nc.scalar.dma_start(orow, u_sb[:])

            else:
                # Final pair: a2 completes first, then a0, a1; a3 runs
                # h-major so its first half can be copied + DMA'd while the
                # second half's matmuls run.  Copies split vector/scalar so
                # neither engine's chain extends past the last matmul by
                # more than one half-copy; out cfgs spread over sync (3) and
                # pool (3) so no sequencer serializes more than ~3 cfgs.
                order = ((0, 2), (0, 0), (1, 2), (1, 0), (0, 1), (1, 1))
                psu_by_a = {}
                u_by_a = {}
                for j, a in order:
                    if j == 0:
                        psu_by_a[a] = psup.tile([128, C], f32, tag="psu",
                                                name=f"psu_{p}_{a}")
                    psu = psu_by_a[a]
                    for h in range(2):
                        nc.tensor.matmul(
                            psu[:, h * 512: (h + 1) * 512],
                            z_sb[:, j, a * 128: (a + 1) * 128],
                            wu_sb[:, j * 1024 + h * 512:
                                  j * 1024 + h * 512 + 512],
                            start=(j == 0), stop=(j == KD - 1),
                            skip_group_check=True,
                        )
                    if j == KD - 1:
                        u_sb = upool.tile([128, C], out_dt, tag="u",
                                          name=f"u_last_{a}")
                        u_by_a[a] = u_sb
                        orow = out[m, b, a * 128:(a + 1) * 128, :]
                        if a == 2:
                            nc.vector.tensor_copy(u_sb[:], psu[:])
                            nc.sync.dma_start(orow, u_sb[:])
                        elif a == 0:
                            nc.scalar.activation(u_sb[:], psu[:], copy_fn)
                            nc.scalar.dma_start(orow, u_sb[:])
                        else:  # a == 1: halves on both engines
                            nc.vector.tensor_copy(u_sb[:, :512], psu[:, :512])
                            nc.sync.dma_start(orow[:, :512], u_sb[:, :512])
                            nc.scalar.activation(u_sb[:, 512:], psu[:, 512:],
                                                 copy_fn)
                            nc.scalar.dma_start(orow[:, 512:], u_sb[:, 512:])
                # a3: h-major accumulation so h0 completes 2 matmuls early.
                # Its PSUM comes from the pszp pool (free once the silus are
                # done) so the a3 matmuls never wait on an output copy.
                for h in range(2):
                    psu3 = pszp.tile([128, 512], f32, tag="psz",
                                     name=f"psu3h{h}")
                    for j in range(KD):
                        nc.tensor.matmul(
                            psu3[:],
                            z_sb[:, j, 3 * 128: 4 * 128],
                            wu_sb[:, j * 1024 + h * 512:
                                  j * 1024 + h * 512 + 512],
                            start=(j == 0), stop=(j == KD - 1),
                        )
                    u3 = upool.tile([128, C // 2], out_dt, tag="u",
                                    name=f"u_last_3{h}")
                    orow3 = out[m, b, 3 * 128: 4 * 128, :]
                    if h == 0:
                        nc.vector.tensor_copy(u3[:], psu3[:])
                        nc.sync.dma_start(orow3[:, :512], u3[:])
                    else:
                        nc.scalar.activation(u3[:], psu3[:], copy_fn)
                        nc.scalar.dma_start(orow3[:, 512:], u3[:])

    nc.compile()
    return nc


def _get_nc():
    if "nc" not in _cache:
        _cache["nc"] = _build()
    return _cache["nc"]


def kernel(x, expert_index, down_w, down_b, up_w):
    global last_results
    import ml_dtypes
    from concourse import bass_utils

    x = np.asarray(x, dtype=np.float32)
    idx = np.asarray(expert_index)
    r = np.arange(M)[:, None]
    wd = np.asarray(down_w, dtype=np.float32)[r, idx]   # [M, B, C, D]
    bd = np.asarray(down_b, dtype=np.float32)[r, idx]   # [M, B, D]
    wu = np.asarray(up_w, dtype=np.float32)[r, idx]     # [M, B, D, C]

    # Pack into SBUF partition-major layouts (see _build comments).
    xt = x.transpose(0, 2, 1).reshape(B, 2, KC // 2, 128, S)
    xt = xt.transpose(0, 1, 3, 2, 4).reshape(B, 2, 128, KC * S // 2)
    wdp = wd.reshape(M, B, KC, 128, D).transpose(0, 1, 3, 2, 4)
    wdp = wdp.reshape(M, B, 128, KC * D)
    wup = wu.reshape(M, B, KD, 128, C).transpose(0, 1, 3, 2, 4)
    wup = wup.reshape(M, B, 128, KD * C)
    bdp = bd.reshape(M, B, KD, 128).transpose(3, 0, 1, 2)  # [128, M, B, KD]

    in_dt = ml_dtypes.bfloat16

    in_maps = []
    for i in range(N_CORES):
        bs = slice(i * B_LOC, (i + 1) * B_LOC)
        # bias rows padded to 576B (see _build): cols 0:16 real, rest zero
        bias_pad = np.zeros((128, 144), dtype=np.float32)
        bias_pad[:, :M * B_LOC * KD] = \
            bdp[:, :, bs, :].reshape(128, M * B_LOC * KD)
        in_maps.append({
            "xtp": np.ascontiguousarray(xt[bs].astype(in_dt)),
            "wdp": np.ascontiguousarray(wdp[:, bs].astype(in_dt)),
            "wup": np.ascontiguousarray(wup[:, bs].astype(in_dt)),
            "bdp": bias_pad,
        })

    nc = _get_nc()
    res = None
    for attempt in range(3):
        try:
            res = bass_utils.run_bass_kernel_spmd(nc, in_maps,
                                                  core_ids=list(range(N_CORES)))
            break
        except Exception:
            # transient NRT_EXEC_UNIT_UNRECOVERABLE device hiccups recover
            # after a short wait; re-raise if persistent
            if attempt == 2:
                raise
            import time
            time.sleep(15)
    last_results = res

    full = np.empty((M, B, S, C), dtype=np.float32)
    for i in range(N_CORES):
        full[:, i * B_LOC:(i + 1) * B_LOC] = np.asarray(
            res.results[i]["out"]).astype(np.float32)
    return full
